# revision 39
# baseline (speedup 1.0000x reference)
"""Trainium2 Bass kernel for the BKT (multi-HMM knowledge tracing) forward model.

Strategy: data-parallel over students (1024 students / 8 cores = 128 per core,
one SBUF partition per student). The T=500 time recursion runs locally per core.

Per-core algebra per step t (all tables precomputed on host):
    c      = A[kc[:,t]]                          [128,100]  (gathered, rows sum to 1)
    G4     = (A @ log_t)[kc[:,t]]                [128,4]    (s,s')  -- in same gather row
    L4     = log_obs[problem[:,t]] in (o,s)      [128,4]
    OLL    = log_obs[problem[:,t]][:, corr]      [128,2]    (s')    -- corr folded into idx
    a2_s   = sum_k c * alpha_s
    t010   = [G4 + OLL(s'), L4, 0, 0] + a2-dup   [128,10]
    e10    = exp(t010); ps5 = pairsum(e10) = [se0,se1,po0,po1,q]
    lg5    = ln(ps5): a3 = lg5[:,0:2], log_py = lg5[:,2:4] - lg5[:,4:5]
    alpha_s' = alpha_s - c * (alpha_s - a3_s)

Critical-path restructure (v-trick): substituting the alpha update into the
next step's a2 gives

    a2_{i+1,s} = w_{i+1,s} + v_i * a3_{i,s}
    w_{i+1,s}  = sum_k u_i,k * alpha_{i,k,s}      u_i = c_{i+1} (1 - c_i)
    v_i        = sum_k c_{i+1,k} c_{i,k} = 1 - sum_k u_i,k   (rows sum to 1)

so the only on-chain per-step ops are
    t010_{i+1} = tabw_{i+1} + (v_i * a3_i)dup     (one DVE stt)
    exp -> pairsum -> ln                          (ACT, DVE, ACT)
while the alpha update (fused custom-DVE lerp, one instruction per s), the
w dots, z/u/v prep and the tabw = tab10 + w-dup prep run off the critical
chain, interleaved into the exp/ln wait windows of the DVE program order
(lerps + z/u fill the exp window; w dots + tabw fill the ln window).

Everything bulky is fp16: gather tables (256B rows, halves SWDGE traffic),
the alpha state, and the c/u/z operands; accumulators, the exp/ln chain and
outputs stay f32 (end-to-end rel err ~1.3e-3 vs the f32 reference).

M4 = G4 + OLL prep runs per-chunk on the gpsimd engine into an f32 chunk
buffer; gathers are issued 3 chunks ahead on 2 SWDGE queues with an
enlarged descriptor ring so slab recycling never stalls the compute
engines. TimelineSim: 1.379 ms (baseline) -> 0.659 ms.
"""

import os
from contextlib import ExitStack

import numpy as np

N_PROBLEMS = 10000
N_KCS = 100
BATCH = 1024
T_FULL = 500
N_CORES = 8
BL = BATCH // N_CORES  # 128 students per core

_CHUNK = 8  # time steps per gather slab


def _log_softmax(x, axis):
    x = x.astype(np.float32)
    m = x.max(axis=axis, keepdims=True)
    e = np.exp(x - m)
    return (x - m) - np.log(e.sum(axis=axis, keepdims=True))


def _wrap_idx(flat):
    """dma_gather index layout: flat index i lives at partition i%16, col i//16,
    replicated across the 8 gpsimd cores (16-partition groups)."""
    assert flat.size % 16 == 0
    w = flat.astype(np.int16).reshape(-1, 16).T  # [16, N/16]
    return np.tile(w, (8, 1))  # [128, N/16]


def _host_tables(A, trans_logits, obs_logits_problem, init_logits):
    P = A.shape[0]
    K = trans_logits.shape[0]
    log_t = _log_softmax(trans_logits, axis=1)  # [K,2,2] normalized over middle axis
    G = A.astype(np.float32) @ log_t.reshape(K, 4)  # [P,4] laid out (s,s')
    L = _log_softmax(obs_logits_problem, axis=2)  # [P,2,2] normalized over outputs

    taba = np.zeros((P, 128), np.float16)
    taba[:, 0:100] = A.astype(np.float16)
    taba[:, 100:104] = G.astype(np.float16)

    # tabp row (2p + corr): [M4-scratch | L4 in (o,s) order | zeros | OLL2]
    # cols 0:4 are overwritten on-chip with M4 = G4 + OLL(s'), making
    # cols 0:10 = [M4, L4, Z2] the contiguous input of the t010 add.
    tabp = np.zeros((2 * P, 128), np.float16)
    L4 = np.stack([L[:, 0, 0], L[:, 1, 0], L[:, 0, 1], L[:, 1, 1]], axis=1)
    tabp[0::2, 4:8] = L4
    tabp[1::2, 4:8] = L4
    tabp[0::2, 10:12] = L[:, :, 0]
    tabp[1::2, 10:12] = L[:, :, 1]

    la0 = _log_softmax(init_logits, axis=1)  # [K,2]
    alpha0 = np.empty((BL, 2 * K), np.float16)
    alpha0[:, 0:K] = la0[:, 0]  # s=0 block
    alpha0[:, K:] = la0[:, 1]  # s=1 block
    return taba, tabp, alpha0


def _setup_act_tables():
    """Both Exp and Ln live in the 'natural_log_exp_and_others' ACT table
    set, but the default set ordering makes bacc pick a different set for
    each, inserting a ~2.7us ACT_TABLE_LOAD per activation (2 per time
    step!). Reorder the set list so that set comes first for both bacc's
    chooser and walrus (via BASS_ACT_ROOT_JSON_PATH), collapsing the loads
    to one for the whole kernel."""
    import glob
    import json
    import tempfile

    if os.environ.get("_BKT_ACT_TABLES"):
        return
    from neuronxcc.driver.Job import Job  # pyright: ignore[reportMissingImports]
    from neuronxcc.driver.jobs.support.FindActInfo import (  # pyright: ignore[reportMissingImports]
        findActInfoFile,
    )

    src = findActInfoFile(Job.getPackageDir(), "gen3")
    d = json.load(open(src))
    d["act_func_sets"] = sorted(
        d["act_func_sets"],
        key=lambda s: s["name"] != "natural_log_exp_and_others")
    tmp = tempfile.mkdtemp(prefix="bkt_act_")
    with open(tmp + "/act_info.json", "w") as f:
        json.dump(d, f)
    for p in glob.glob(os.path.dirname(src) + "/*"):
        b = os.path.basename(p)
        if b != "act_info.json":
            os.symlink(p, tmp + "/" + b)
    os.environ["BASS_ACT_ROOT_JSON_PATH"] = tmp + "/act_info.json"
    os.environ["_BKT_ACT_TABLES"] = "1"

    import concourse.bacc as bacc_mod
    import concourse.mybir as mybir

    def tables(arch):
        return {
            e["name"]: {mybir.ActivationFunctionType.from_pwp(v)
                        for v in e["act"].keys()}
            for e in d["act_func_sets"]
        }

    bacc_mod.get_activation_tables = tables


_LERP_OP = None


def _register_lerp():
    """Custom DVE op: out = in0 + in1*(s0 - in0)  (per-partition scalar s0).
    One instruction for the per-(k,s) blend alpha' = (1-c)*alpha + c*a3."""
    global _LERP_OP
    if _LERP_OP is not None:
        return _LERP_OP
    import concourse.dve_ops as dops
    from concourse.dve_spec import C0, Spec, Src0, Src1, _has_src1, lower
    from concourse.dve_table_gen import dve_ver_for
    from concourse.dve_uop import DveOpSpec

    name = "LERP_BLEND_ANT"
    for o in dops.OPS:
        if o.name == name:
            _LERP_OP = o
            return o
    spec = Spec(
        body=Src0 + Src1 * (C0 - Src0),
        reference=lambda in0, in1, s0, s1, imm2: (
            in0.astype(np.float32) + in1 * (s0 - in0)
        ).astype(np.float32),
    )
    row = max(dops._SUB_OPCODE_FOR_NAME.values()) + 1
    assert row < 0x20
    dops._SUB_OPCODE_FOR_NAME[name] = row
    ver = dve_ver_for("TRN2")
    uops = lower(spec, ver=ver)
    sha = DveOpSpec(
        name=name, opcode=row, uops=uops, rd1_en=_has_src1(spec)
    ).sha(ver)
    op = dops.DveOp(name, spec, subdim=False, uops_sha={ver: sha},
                    perf_en={ver: True})
    dops.OPS.append(op)
    dops.CUSTOM_DVE_SPECS[name] = spec
    _LERP_OP = op
    return op


def _emit_program(T, Tc):
    import concourse.mybir as mybir
    import concourse.tile as tile
    from concourse import bacc

    _setup_act_tables()
    lerp_op = _register_lerp()

    f32 = mybir.dt.float32
    f16 = mybir.dt.float16
    i16 = mybir.dt.int16
    Alu = mybir.AluOpType
    Act = mybir.ActivationFunctionType
    K = N_KCS

    nc = bacc.Bacc("TRN2", target_bir_lowering=False, debug=False,
                   dynamic_dma_scratch_size=65536, num_swdge_queues=2)

    taba = nc.dram_tensor("taba", [N_PROBLEMS, 128], f16, kind="ExternalInput")
    tabp = nc.dram_tensor("tabp", [2 * N_PROBLEMS, 128], f16, kind="ExternalInput")
    kcw = nc.dram_tensor("kcw", [128, T * 8], i16, kind="ExternalInput")
    ppw = nc.dram_tensor("ppw", [128, T * 8], i16, kind="ExternalInput")
    alpha0 = nc.dram_tensor("alpha0", [BL, 2 * K], f16, kind="ExternalInput")
    out = nc.dram_tensor("out", [BL, T * 2], f32, kind="ExternalOutput")

    assert Tc * 128 <= 1024
    chunks = []  # (t0, tcn)
    t0 = 0
    while t0 < T:
        chunks.append((t0, min(Tc, T - t0)))
        t0 += Tc
    n_chunks = len(chunks)

    from concourse import library_config

    with ExitStack() as ctx:
        tc = ctx.enter_context(tile.TileContext(nc))
        nc.gpsimd.load_library(library_config.mlp)
        idx_pool = ctx.enter_context(tc.tile_pool(name="idx", bufs=1))
        slab_pool = ctx.enter_context(tc.tile_pool(name="slabs", bufs=6))
        state_pool = ctx.enter_context(tc.tile_pool(name="state", bufs=2))
        small_pool = ctx.enter_context(tc.tile_pool(name="small", bufs=4))
        u_pool = ctx.enter_context(tc.tile_pool(name="u", bufs=3))
        out_pool = ctx.enter_context(tc.tile_pool(name="outb", bufs=1))


        kcw_t = idx_pool.tile([128, T * 8], i16, tag="kcw")
        nc.sync.dma_start(kcw_t[:], kcw.ap())
        ppw_t = idx_pool.tile([128, T * 8], i16, tag="ppw")
        nc.sync.dma_start(ppw_t[:], ppw.ap())

        alpha = state_pool.tile([128, 2 * K], f16, tag="alpha")
        nc.sync.dma_start(alpha[:], alpha0.ap())

        outbuf = out_pool.tile([128, T * 2], f32)
        # per-step ln() results land here: [se0, se1, po0, po1, q] per t
        lgbuf = out_pool.tile([128, T * 5], f32)

        slabsA = [None] * n_chunks
        slabsP = [None] * n_chunks
        ni_regs = {}
        for tcn in sorted({c[1] for c in chunks}):
            r = nc.gpsimd.alloc_register(f"ni{tcn}")
            nc.gpsimd.reg_mov(r, tcn * 128)
            ni_regs[tcn] = r

        def issue_gather(n):
            t0, tcn = chunks[n]
            ni = ni_regs[tcn]
            sa = slab_pool.tile([128, Tc, 128], f16, tag="slabA")
            nc.gpsimd.dma_gather(
                sa[:, 0:tcn, :], taba.ap(), kcw_t[:, t0 * 8:(t0 + tcn) * 8],
                num_idxs=tcn * 128, num_idxs_reg=ni, elem_size=128,
                queue_num=0,
            )
            sp = slab_pool.tile([128, Tc, 128], f16, tag="slabP")
            nc.gpsimd.dma_gather(
                sp[:, 0:tcn, :], tabp.ap(), ppw_t[:, t0 * 8:(t0 + tcn) * 8],
                num_idxs=tcn * 128, num_idxs_reg=ni, elem_size=128,
                queue_num=1,
            )
            slabsA[n], slabsP[n] = sa, sp

        for _n in range(min(3, n_chunks)):
            issue_gather(_n)

        def c_ap(t):
            return slabsA[t // Tc][:, t % Tc, 0:K]

        tabfs = [None] * n_chunks

        def tab10_ap(t):
            return tabfs[t // Tc][:, t % Tc, :]

        def prep_m4(n):
            """Build the f32 per-chunk tab10 buffer: M4 = G4 + OLL into
            cols 0:4 (two 3D adds, one per s), then L4/zeros converted from
            the f16 slab into cols 4:10."""
            t0, tcn = chunks[n]
            sa, sp = slabsA[n], slabsP[n]
            tabf = slab_pool.tile([128, Tc, 10], f32, tag="tabf", name="tabf")
            tabfs[n] = tabf
            for s in range(2):
                nc.gpsimd.tensor_tensor(
                    out=tabf[:, 0:tcn, 2 * s:2 * s + 2],
                    in0=sa[:, 0:tcn, 100 + 2 * s:102 + 2 * s],
                    in1=sp[:, 0:tcn, 10:12],
                    op=Alu.add,
                )
            nc.gpsimd.tensor_copy(
                out=tabf[:, 0:tcn, 4:10], in_=sp[:, 0:tcn, 4:10],
            )

        prep_m4(0)

        # rings for per-step intermediates
        def small(w, tag):
            st = small_pool.tile([128, w], f32, tag=tag, name=tag)
            return st


        # prologue for step 0: w0_s = sum_k c_0 * alpha0_s ; u_0, vacc_0
        wt = small(2, "w")
        for s in range(2):
            scr = u_pool.tile([128, K], f16, tag="scr")
            nc.vector.scalar_tensor_tensor(
                out=scr[:], in0=c_ap(0), scalar=0.0,
                in1=alpha[:, s * K:(s + 1) * K],
                op0=Alu.bypass, op1=Alu.mult,
                accum_out=wt[:, s:s + 1],
            )
        tabw = small(10, "tabw")
        nc.vector.tensor_tensor(
            out=tabw[:].rearrange("p (a b) -> p a b", b=2),
            in0=tab10_ap(0).rearrange("p (a b) -> p a b", b=2),
            in1=wt[:].unsqueeze(1).broadcast_to([128, 5, 2]),
            op=Alu.add,
        )
        if T > 1:
            zt = u_pool.tile([128, K], f16, tag="z")
            ut_next = u_pool.tile([128, K], f16, tag="u")
            vt_next = small(1, "v")
            nc.vector.scalar_tensor_tensor(
                out=zt[:], in0=c_ap(0), scalar=0.0, in1=c_ap(1),
                op0=Alu.bypass, op1=Alu.mult,
                accum_out=vt_next[:],
            )
            nc.vector.tensor_tensor(
                out=ut_next[:], in0=c_ap(1), in1=zt[:], op=Alu.subtract,
            )

        for n in range(n_chunks):
            if n + 3 < n_chunks:
                issue_gather(n + 3)
            for j in range(chunks[n][1]):
                t = chunks[n][0] + j
                # ---- chain: t010 -> exp -> pairsum -> ln ----
                if t == 0:
                    # t010_0 = tabw_0 (v_{-1}=0): exp tabw directly
                    e_in = tabw
                else:
                    t010 = small(10, "t010")
                    nc.vector.scalar_tensor_tensor(
                        out=t010[:].rearrange("p (a b) -> p a b", b=2),
                        in0=lgbuf[:, 5 * (t - 1):5 * (t - 1) + 2]
                            .unsqueeze(1).broadcast_to([128, 5, 2]),
                        scalar=vt_prev[:, 0:1],
                        in1=tabw[:].rearrange("p (a b) -> p a b", b=2),
                        op0=Alu.mult, op1=Alu.add,
                    )
                    e_in = t010
                e10 = small(10, "e10")
                nc.scalar.activation(e10[:], e_in[:], Act.Exp)

                # ---- off-chain A (runs on DVE while ACT does exp_t):
                # alpha_{t} update from a3_{t-1}, then w_{t+1} dots
                if t >= 1:
                    a3prev = lgbuf[:, 5 * (t - 1):5 * (t - 1) + 2]
                    alpha_new = state_pool.tile([128, 2 * K], f16, tag="alpha")
                    nc.vector._custom_dve(
                        lerp_op,
                        out=alpha_new[:, 0:K],
                        in0=alpha[:, 0:K],
                        in1=c_ap(t - 1),
                        s0=a3prev[:, 0:1],
                    )
                # ---- chain: pairsum -> ln ----
                if t >= 1:
                    nc.vector._custom_dve(
                        lerp_op,
                        out=alpha_new[:, K:2 * K],
                        in0=alpha[:, K:2 * K],
                        in1=c_ap(t - 1),
                        s0=a3prev[:, 1:2],
                    )
                    alpha = alpha_new
                vt_prev = vt_next
                ut_prev = ut_next
                if t + 2 < T:
                    zt = u_pool.tile([128, K], f16, tag="z")
                    ut_next = u_pool.tile([128, K], f16, tag="u")
                    vt_next = small(1, "v")
                    nc.vector.scalar_tensor_tensor(
                        out=zt[:], in0=c_ap(t + 1), scalar=0.0,
                        in1=c_ap(t + 2),
                        op0=Alu.bypass, op1=Alu.mult,
                        accum_out=vt_next[:],
                    )
                    nc.vector.tensor_tensor(
                        out=ut_next[:], in0=c_ap(t + 2), in1=zt[:], op=Alu.subtract,
                    )
                ps5 = small(5, "ps5")
                ev = e10[:].rearrange("p (a b) -> p a b", b=2)
                nc.vector.tensor_tensor(
                    out=ps5[:], in0=ev[:, :, 0], in1=ev[:, :, 1], op=Alu.add,
                )
                lg5 = lgbuf[:, 5 * t:5 * t + 5]
                nc.scalar.activation(lg5, ps5[:], Act.Ln)

                # ---- off-chain B: w dots, tabw_{t+1}, u/v_{t+1} ----
                if t + 1 < T:
                    wt = small(2, "w")
                    for s in range(2):
                        scr = u_pool.tile([128, K], f16, tag="scr")
                        nc.vector.scalar_tensor_tensor(
                            out=scr[:], in0=ut_prev[:], scalar=0.0,
                            in1=alpha[:, s * K:(s + 1) * K],
                            op0=Alu.bypass, op1=Alu.mult,
                            accum_out=wt[:, s:s + 1],
                        )
                if t + 1 < T:
                    tabw = small(10, "tabw")
                    nc.vector.tensor_tensor(
                        out=tabw[:].rearrange("p (a b) -> p a b", b=2),
                        in0=tab10_ap(t + 1).rearrange("p (a b) -> p a b", b=2),
                        in1=wt[:].unsqueeze(1).broadcast_to([128, 5, 2]),
                        op=Alu.add,
                    )
                if j == chunks[n][1] - 2 and n + 1 < n_chunks:
                    prep_m4(n + 1)

        # normalize all outputs at once: log_py[t, o] = lpo[t, o] - lq[t]
        lg3 = lgbuf[:].rearrange("p (t f) -> p t f", f=5)
        nc.vector.tensor_tensor(
            out=outbuf[:].rearrange("p (t o) -> p t o", o=2),
            in0=lg3[:, :, 2:4],
            in1=lg3[:, :, 4:5].broadcast_to([128, T, 2]),
            op=Alu.subtract,
        )
        nc.sync.dma_start(out.ap(), outbuf[:])

    nc.compile()
    return nc


def _prep_inputs(corr, kc, problem, A, trans_logits, obs_logits_problem, init_logits, T):
    corr = np.asarray(corr).astype(np.int64)
    kc = np.asarray(kc).astype(np.int64)
    problem = np.asarray(problem).astype(np.int64)
    taba, tabp, alpha0 = _host_tables(
        np.asarray(A), np.asarray(trans_logits),
        np.asarray(obs_logits_problem), np.asarray(init_logits))

    in_maps = []
    for i in range(N_CORES):
        sl = slice(i * BL, (i + 1) * BL)
        kc_l = kc[sl, :T]  # [128, T]
        pp_l = 2 * problem[sl, :T] + corr[sl, :T]
        kcw = _wrap_idx(kc_l.T.ravel())
        ppw = _wrap_idx(pp_l.T.ravel())
        in_maps.append({
            "taba": taba, "tabp": tabp, "kcw": kcw, "ppw": ppw,
            "alpha0": alpha0,
        })
    return in_maps


def kernel(corr, kc, problem, A, trans_logits, obs_logits_problem, init_logits,
           _T=None, _trace=False):
    T = _T or T_FULL
    nc = _emit_program(T, min(_CHUNK, T))
    in_maps = _prep_inputs(corr, kc, problem, A, trans_logits,
                           obs_logits_problem, init_logits, T)

    from concourse.bass_utils import run_bass_kernel_spmd
    res = run_bass_kernel_spmd(nc, in_maps, core_ids=list(range(N_CORES)),
                               trace=_trace)
    outs = [r["out"].reshape(BL, T, 2) for r in res.results]
    full = np.concatenate(outs, axis=0).astype(np.float32)
    kernel.last_results = res
    return full


if __name__ == "__main__":
    pass


# revision 42
# speedup vs baseline: 1.0106x; 1.0106x over previous
"""Trainium2 Bass kernel for the BKT (multi-HMM knowledge tracing) forward model.

Strategy: data-parallel over students (1024 students / 8 cores = 128 per core,
one SBUF partition per student). The T=500 time recursion runs locally per core.

Per-core algebra per step t (all tables precomputed on host):
    c      = A[kc[:,t]]                          [128,100]  (gathered, rows sum to 1)
    G4     = (A @ log_t)[kc[:,t]]                [128,4]    (s,s')  -- in same gather row
    L4     = log_obs[problem[:,t]] in (o,s)      [128,4]
    OLL    = log_obs[problem[:,t]][:, corr]      [128,2]    (s')    -- corr folded into idx
    a2_s   = sum_k c * alpha_s
    t010   = [G4 + OLL(s'), L4, 0, 0] + a2-dup   [128,10]
    e10    = exp(t010); ps5 = pairsum(e10) = [se0,se1,po0,po1,q]
    lg5    = ln(ps5): a3 = lg5[:,0:2], log_py = lg5[:,2:4] - lg5[:,4:5]
    alpha_s' = alpha_s - c * (alpha_s - a3_s)

Critical-path restructure (v-trick): substituting the alpha update into the
next step's a2 gives

    a2_{i+1,s} = w_{i+1,s} + v_i * a3_{i,s}
    w_{i+1,s}  = sum_k u_i,k * alpha_{i,k,s}      u_i = c_{i+1} (1 - c_i)
    v_i        = sum_k c_{i+1,k} c_{i,k} = 1 - sum_k u_i,k   (rows sum to 1)

so the only on-chain per-step ops are
    t010_{i+1} = tabw_{i+1} + (v_i * a3_i)dup     (one DVE stt)
    exp -> pairsum -> ln                          (ACT, DVE, ACT)
while the alpha update (fused custom-DVE lerp, one instruction per s), the
w dots, z/u/v prep and the tabw = tab10 + w-dup prep run off the critical
chain, interleaved into the exp/ln wait windows of the DVE program order
(lerps + z/u fill the exp window; w dots + tabw fill the ln window).

Everything bulky is fp16: gather tables (256B rows, halves SWDGE traffic),
the alpha state, and the c/u/z operands; accumulators, the exp/ln chain and
outputs stay f32 (end-to-end rel err ~1.3e-3 vs the f32 reference).

M4 = G4 + OLL prep runs per-chunk on the gpsimd engine into an f32 chunk
buffer; gathers are issued 3 chunks ahead on 2 SWDGE queues with an
enlarged descriptor ring so slab recycling never stalls the compute
engines. TimelineSim: 1.379 ms (baseline) -> 0.659 ms.
"""

import os
from contextlib import ExitStack

import numpy as np

N_PROBLEMS = 10000
N_KCS = 100
BATCH = 1024
T_FULL = 500
N_CORES = 8
BL = BATCH // N_CORES  # 128 students per core

_CHUNK = 8  # time steps per gather slab


def _log_softmax(x, axis):
    x = x.astype(np.float32)
    m = x.max(axis=axis, keepdims=True)
    e = np.exp(x - m)
    return (x - m) - np.log(e.sum(axis=axis, keepdims=True))


def _wrap_idx(flat):
    """dma_gather index layout: flat index i lives at partition i%16, col i//16,
    replicated across the 8 gpsimd cores (16-partition groups)."""
    assert flat.size % 16 == 0
    w = flat.astype(np.int16).reshape(-1, 16).T  # [16, N/16]
    return np.tile(w, (8, 1))  # [128, N/16]


def _host_tables(A, trans_logits, obs_logits_problem, init_logits):
    P = A.shape[0]
    K = trans_logits.shape[0]
    log_t = _log_softmax(trans_logits, axis=1)  # [K,2,2] normalized over middle axis
    G = A.astype(np.float32) @ log_t.reshape(K, 4)  # [P,4] laid out (s,s')
    L = _log_softmax(obs_logits_problem, axis=2)  # [P,2,2] normalized over outputs

    taba = np.zeros((P, 128), np.float16)
    taba[:, 0:100] = A.astype(np.float16)
    taba[:, 100:104] = G.astype(np.float16)

    # tabp row (2p + corr): [M4-scratch | L4 in (o,s) order | zeros | OLL2]
    # cols 0:4 are overwritten on-chip with M4 = G4 + OLL(s'), making
    # cols 0:10 = [M4, L4, Z2] the contiguous input of the t010 add.
    tabp = np.zeros((2 * P, 128), np.float16)
    L4 = np.stack([L[:, 0, 0], L[:, 1, 0], L[:, 0, 1], L[:, 1, 1]], axis=1)
    tabp[0::2, 4:8] = L4
    tabp[1::2, 4:8] = L4
    tabp[0::2, 10:12] = L[:, :, 0]
    tabp[1::2, 10:12] = L[:, :, 1]

    la0 = _log_softmax(init_logits, axis=1)  # [K,2]
    alpha0 = np.empty((BL, 2 * K), np.float16)
    alpha0[:, 0:K] = la0[:, 0]  # s=0 block
    alpha0[:, K:] = la0[:, 1]  # s=1 block
    return taba, tabp, alpha0


def _setup_act_tables():
    """Both Exp and Ln live in the 'natural_log_exp_and_others' ACT table
    set, but the default set ordering makes bacc pick a different set for
    each, inserting a ~2.7us ACT_TABLE_LOAD per activation (2 per time
    step!). Reorder the set list so that set comes first for both bacc's
    chooser and walrus (via BASS_ACT_ROOT_JSON_PATH), collapsing the loads
    to one for the whole kernel."""
    import glob
    import json
    import tempfile

    if os.environ.get("_BKT_ACT_TABLES"):
        return
    from neuronxcc.driver.Job import Job  # pyright: ignore[reportMissingImports]
    from neuronxcc.driver.jobs.support.FindActInfo import (  # pyright: ignore[reportMissingImports]
        findActInfoFile,
    )

    src = findActInfoFile(Job.getPackageDir(), "gen3")
    d = json.load(open(src))
    d["act_func_sets"] = sorted(
        d["act_func_sets"],
        key=lambda s: s["name"] != "natural_log_exp_and_others")
    tmp = tempfile.mkdtemp(prefix="bkt_act_")
    with open(tmp + "/act_info.json", "w") as f:
        json.dump(d, f)
    for p in glob.glob(os.path.dirname(src) + "/*"):
        b = os.path.basename(p)
        if b != "act_info.json":
            os.symlink(p, tmp + "/" + b)
    os.environ["BASS_ACT_ROOT_JSON_PATH"] = tmp + "/act_info.json"
    os.environ["_BKT_ACT_TABLES"] = "1"

    import concourse.bacc as bacc_mod
    import concourse.mybir as mybir

    def tables(arch):
        return {
            e["name"]: {mybir.ActivationFunctionType.from_pwp(v)
                        for v in e["act"].keys()}
            for e in d["act_func_sets"]
        }

    bacc_mod.get_activation_tables = tables


_LERP_OP = None
_ACA_OP = None


def _register_aca():
    """Custom DVE op: out = in0*(1 - s0) + in1. Lets the chain's t010 stt
    consume the affine_mul_reduce accumulator (1 - v) directly, removing
    the separate v-flip / u-product instructions."""
    global _ACA_OP
    if _ACA_OP is not None:
        return _ACA_OP
    import concourse.dve_ops as dops
    from concourse.dve_spec import C0, Spec, Src0, Src1, _has_src1, lower
    from concourse.dve_table_gen import dve_ver_for
    from concourse.dve_uop import DveOpSpec

    name = "ADD_COMPL_MUL_ANT"
    for o in dops.OPS:
        if o.name == name:
            _ACA_OP = o
            return o
    def _aca_ref(in0, in1, s0, s1, imm2):
        p = in0.shape[0]
        a = in0.astype(np.float32).reshape(p, -1)
        b = np.asarray(in1, dtype=np.float32).reshape(p, -1)
        s = np.asarray(s0, dtype=np.float32).reshape(p, -1)
        return ((a + b) - a * s).astype(np.float32)

    spec = Spec(
        body=(Src0 + Src1) - Src0 * C0,
        reference=_aca_ref,
    )
    row = max(dops._SUB_OPCODE_FOR_NAME.values()) + 1
    assert row < 0x20
    dops._SUB_OPCODE_FOR_NAME[name] = row
    ver = dve_ver_for("TRN2")
    uops = lower(spec, ver=ver)
    sha = DveOpSpec(
        name=name, opcode=row, uops=uops, rd1_en=_has_src1(spec)
    ).sha(ver)
    op = dops.DveOp(name, spec, subdim=False, uops_sha={ver: sha},
                    perf_en={ver: True})
    dops.OPS.append(op)
    dops.CUSTOM_DVE_SPECS[name] = spec
    _ACA_OP = op
    return op


def _register_lerp():
    """Custom DVE op: out = in0 + in1*(s0 - in0)  (per-partition scalar s0).
    One instruction for the per-(k,s) blend alpha' = (1-c)*alpha + c*a3."""
    global _LERP_OP
    if _LERP_OP is not None:
        return _LERP_OP
    import concourse.dve_ops as dops
    from concourse.dve_spec import C0, Spec, Src0, Src1, _has_src1, lower
    from concourse.dve_table_gen import dve_ver_for
    from concourse.dve_uop import DveOpSpec

    name = "LERP_BLEND_ANT"
    for o in dops.OPS:
        if o.name == name:
            _LERP_OP = o
            return o
    spec = Spec(
        body=Src0 + Src1 * (C0 - Src0),
        reference=lambda in0, in1, s0, s1, imm2: (
            in0.astype(np.float32) + in1 * (s0 - in0)
        ).astype(np.float32),
    )
    row = max(dops._SUB_OPCODE_FOR_NAME.values()) + 1
    assert row < 0x20
    dops._SUB_OPCODE_FOR_NAME[name] = row
    ver = dve_ver_for("TRN2")
    uops = lower(spec, ver=ver)
    sha = DveOpSpec(
        name=name, opcode=row, uops=uops, rd1_en=_has_src1(spec)
    ).sha(ver)
    op = dops.DveOp(name, spec, subdim=False, uops_sha={ver: sha},
                    perf_en={ver: True})
    dops.OPS.append(op)
    dops.CUSTOM_DVE_SPECS[name] = spec
    _LERP_OP = op
    return op


def _emit_program(T, Tc):
    import concourse.mybir as mybir
    import concourse.tile as tile
    from concourse import bacc

    _setup_act_tables()
    lerp_op = _register_lerp()
    aca_op = _register_aca()

    f32 = mybir.dt.float32
    f16 = mybir.dt.float16
    i16 = mybir.dt.int16
    Alu = mybir.AluOpType
    Act = mybir.ActivationFunctionType
    K = N_KCS

    nc = bacc.Bacc("TRN2", target_bir_lowering=False, debug=False,
                   dynamic_dma_scratch_size=65536, num_swdge_queues=2)

    taba = nc.dram_tensor("taba", [N_PROBLEMS, 128], f16, kind="ExternalInput")
    tabp = nc.dram_tensor("tabp", [2 * N_PROBLEMS, 128], f16, kind="ExternalInput")
    kcw = nc.dram_tensor("kcw", [128, T * 8], i16, kind="ExternalInput")
    ppw = nc.dram_tensor("ppw", [128, T * 8], i16, kind="ExternalInput")
    alpha0 = nc.dram_tensor("alpha0", [BL, 2 * K], f16, kind="ExternalInput")
    out = nc.dram_tensor("out", [BL, T * 2], f32, kind="ExternalOutput")

    assert Tc * 128 <= 1024
    chunks = []  # (t0, tcn)
    t0 = 0
    while t0 < T:
        chunks.append((t0, min(Tc, T - t0)))
        t0 += Tc
    n_chunks = len(chunks)

    from concourse import library_config

    with ExitStack() as ctx:
        tc = ctx.enter_context(tile.TileContext(nc))
        nc.gpsimd.load_library(library_config.mlp)
        idx_pool = ctx.enter_context(tc.tile_pool(name="idx", bufs=1))
        slab_pool = ctx.enter_context(tc.tile_pool(name="slabs", bufs=6))
        state_pool = ctx.enter_context(tc.tile_pool(name="state", bufs=2))
        small_pool = ctx.enter_context(tc.tile_pool(name="small", bufs=4))
        u_pool = ctx.enter_context(tc.tile_pool(name="u", bufs=3))
        out_pool = ctx.enter_context(tc.tile_pool(name="outb", bufs=1))


        kcw_t = idx_pool.tile([128, T * 8], i16, tag="kcw")
        nc.sync.dma_start(kcw_t[:], kcw.ap())
        ppw_t = idx_pool.tile([128, T * 8], i16, tag="ppw")
        nc.sync.dma_start(ppw_t[:], ppw.ap())

        alpha = state_pool.tile([128, 2 * K], f16, tag="alpha")
        nc.sync.dma_start(alpha[:], alpha0.ap())

        outbuf = out_pool.tile([128, T * 2], f32)
        # per-step ln() results land here: [se0, se1, po0, po1, q] per t
        lgbuf = out_pool.tile([128, T * 5], f32)

        slabsA = [None] * n_chunks
        slabsP = [None] * n_chunks
        ni_regs = {}
        for tcn in sorted({c[1] for c in chunks}):
            r = nc.gpsimd.alloc_register(f"ni{tcn}")
            nc.gpsimd.reg_mov(r, tcn * 128)
            ni_regs[tcn] = r

        def issue_gather(n):
            t0, tcn = chunks[n]
            ni = ni_regs[tcn]
            sa = slab_pool.tile([128, Tc, 128], f16, tag="slabA")
            nc.gpsimd.dma_gather(
                sa[:, 0:tcn, :], taba.ap(), kcw_t[:, t0 * 8:(t0 + tcn) * 8],
                num_idxs=tcn * 128, num_idxs_reg=ni, elem_size=128,
                queue_num=0,
            )
            sp = slab_pool.tile([128, Tc, 128], f16, tag="slabP")
            nc.gpsimd.dma_gather(
                sp[:, 0:tcn, :], tabp.ap(), ppw_t[:, t0 * 8:(t0 + tcn) * 8],
                num_idxs=tcn * 128, num_idxs_reg=ni, elem_size=128,
                queue_num=1,
            )
            slabsA[n], slabsP[n] = sa, sp

        for _n in range(min(3, n_chunks)):
            issue_gather(_n)

        def c_ap(t):
            return slabsA[t // Tc][:, t % Tc, 0:K]

        tabfs = [None] * n_chunks

        def tab10_ap(t):
            return tabfs[t // Tc][:, t % Tc, :]

        def prep_m4(n):
            """Build the f32 per-chunk tab10 buffer: M4 = G4 + OLL into
            cols 0:4 (two 3D adds, one per s), then L4/zeros converted from
            the f16 slab into cols 4:10."""
            t0, tcn = chunks[n]
            sa, sp = slabsA[n], slabsP[n]
            tabf = slab_pool.tile([128, Tc, 10], f32, tag="tabf", name="tabf")
            tabfs[n] = tabf
            for s in range(2):
                nc.gpsimd.tensor_tensor(
                    out=tabf[:, 0:tcn, 2 * s:2 * s + 2],
                    in0=sa[:, 0:tcn, 100 + 2 * s:102 + 2 * s],
                    in1=sp[:, 0:tcn, 10:12],
                    op=Alu.add,
                )
            nc.gpsimd.tensor_copy(
                out=tabf[:, 0:tcn, 4:10], in_=sp[:, 0:tcn, 4:10],
            )

        prep_m4(0)

        # rings for per-step intermediates
        def small(w, tag):
            st = small_pool.tile([128, w], f32, tag=tag, name=tag)
            return st


        # prologue for step 0: w0_s = sum_k c_0 * alpha0_s ; u_0, vacc_0
        wt = small(2, "w")
        for s in range(2):
            scr = u_pool.tile([128, K], f16, tag="scr")
            nc.vector.scalar_tensor_tensor(
                out=scr[:], in0=c_ap(0), scalar=0.0,
                in1=alpha[:, s * K:(s + 1) * K],
                op0=Alu.bypass, op1=Alu.mult,
                accum_out=wt[:, s:s + 1],
            )
        tabw = small(10, "tabw")
        nc.vector.tensor_tensor(
            out=tabw[:].rearrange("p (a b) -> p a b", b=2),
            in0=tab10_ap(0).rearrange("p (a b) -> p a b", b=2),
            in1=wt[:].unsqueeze(1).broadcast_to([128, 5, 2]),
            op=Alu.add,
        )
        if T > 1:
            ut_next = u_pool.tile([128, K], f16, tag="u")
            nv_next = small(1, "nv")
            nc.vector.affine_mul_reduce(
                out=ut_next[:], accum_out=nv_next[:],
                in0=c_ap(0), in1=c_ap(1), scale=-1.0, bias=1.0,
            )

        for n in range(n_chunks):
            if n + 3 < n_chunks:
                issue_gather(n + 3)
            for j in range(chunks[n][1]):
                t = chunks[n][0] + j
                # ---- chain: t010 -> exp -> pairsum -> ln ----
                if t == 0:
                    # t010_0 = tabw_0 (v_{-1}=0): exp tabw directly
                    e_in = tabw
                else:
                    t010 = small(10, "t010")
                    nc.vector._custom_dve(
                        aca_op,
                        out=t010[:].rearrange("p (a b) -> p a b", b=2),
                        in0=lgbuf[:, 5 * (t - 1):5 * (t - 1) + 2]
                            .unsqueeze(1).broadcast_to([128, 5, 2]),
                        in1=tabw[:].rearrange("p (a b) -> p a b", b=2),
                        s0=nv_prev[:, 0:1],
                    )
                    e_in = t010
                e10 = small(10, "e10")
                nc.scalar.activation(e10[:], e_in[:], Act.Exp)

                # ---- off-chain A (runs on DVE while ACT does exp_t):
                # alpha_{t} update from a3_{t-1}, then w_{t+1} dots
                if t >= 1:
                    a3prev = lgbuf[:, 5 * (t - 1):5 * (t - 1) + 2]
                    alpha_new = state_pool.tile([128, 2 * K], f16, tag="alpha")
                    nc.vector._custom_dve(
                        lerp_op,
                        out=alpha_new[:, 0:K],
                        in0=alpha[:, 0:K],
                        in1=c_ap(t - 1),
                        s0=a3prev[:, 0:1],
                    )
                # ---- chain: pairsum -> ln ----
                if t >= 1:
                    nc.vector._custom_dve(
                        lerp_op,
                        out=alpha_new[:, K:2 * K],
                        in0=alpha[:, K:2 * K],
                        in1=c_ap(t - 1),
                        s0=a3prev[:, 1:2],
                    )
                    alpha = alpha_new
                nv_prev = nv_next
                ut_prev = ut_next
                if t + 2 < T:
                    ut_next = u_pool.tile([128, K], f16, tag="u")
                    nv_next = small(1, "nv")
                    nc.vector.affine_mul_reduce(
                        out=ut_next[:], accum_out=nv_next[:],
                        in0=c_ap(t + 1), in1=c_ap(t + 2), scale=-1.0, bias=1.0,
                    )
                ps5 = small(5, "ps5")
                ev = e10[:].rearrange("p (a b) -> p a b", b=2)
                nc.vector.tensor_tensor(
                    out=ps5[:], in0=ev[:, :, 0], in1=ev[:, :, 1], op=Alu.add,
                )
                lg5 = lgbuf[:, 5 * t:5 * t + 5]
                nc.scalar.activation(lg5, ps5[:], Act.Ln)

                # ---- off-chain B: w dots, tabw_{t+1}, u/v_{t+1} ----
                if t + 1 < T:
                    wt = small(2, "w")
                    for s in range(2):
                        scr = u_pool.tile([128, K], f16, tag="scr")
                        nc.vector.scalar_tensor_tensor(
                            out=scr[:], in0=ut_prev[:], scalar=0.0,
                            in1=alpha[:, s * K:(s + 1) * K],
                            op0=Alu.bypass, op1=Alu.mult,
                            accum_out=wt[:, s:s + 1],
                        )
                if t + 1 < T:
                    tabw = small(10, "tabw")
                    nc.vector.tensor_tensor(
                        out=tabw[:].rearrange("p (a b) -> p a b", b=2),
                        in0=tab10_ap(t + 1).rearrange("p (a b) -> p a b", b=2),
                        in1=wt[:].unsqueeze(1).broadcast_to([128, 5, 2]),
                        op=Alu.add,
                    )
                if j == chunks[n][1] - 2 and n + 1 < n_chunks:
                    prep_m4(n + 1)

        # normalize all outputs at once: log_py[t, o] = lpo[t, o] - lq[t]
        lg3 = lgbuf[:].rearrange("p (t f) -> p t f", f=5)
        nc.vector.tensor_tensor(
            out=outbuf[:].rearrange("p (t o) -> p t o", o=2),
            in0=lg3[:, :, 2:4],
            in1=lg3[:, :, 4:5].broadcast_to([128, T, 2]),
            op=Alu.subtract,
        )
        nc.sync.dma_start(out.ap(), outbuf[:])

    nc.compile()
    return nc


def _prep_inputs(corr, kc, problem, A, trans_logits, obs_logits_problem, init_logits, T):
    corr = np.asarray(corr).astype(np.int64)
    kc = np.asarray(kc).astype(np.int64)
    problem = np.asarray(problem).astype(np.int64)
    taba, tabp, alpha0 = _host_tables(
        np.asarray(A), np.asarray(trans_logits),
        np.asarray(obs_logits_problem), np.asarray(init_logits))

    in_maps = []
    for i in range(N_CORES):
        sl = slice(i * BL, (i + 1) * BL)
        kc_l = kc[sl, :T]  # [128, T]
        pp_l = 2 * problem[sl, :T] + corr[sl, :T]
        kcw = _wrap_idx(kc_l.T.ravel())
        ppw = _wrap_idx(pp_l.T.ravel())
        in_maps.append({
            "taba": taba, "tabp": tabp, "kcw": kcw, "ppw": ppw,
            "alpha0": alpha0,
        })
    return in_maps


def kernel(corr, kc, problem, A, trans_logits, obs_logits_problem, init_logits,
           _T=None, _trace=False):
    T = _T or T_FULL
    nc = _emit_program(T, min(_CHUNK, T))
    in_maps = _prep_inputs(corr, kc, problem, A, trans_logits,
                           obs_logits_problem, init_logits, T)

    from concourse.bass_utils import run_bass_kernel_spmd
    res = run_bass_kernel_spmd(nc, in_maps, core_ids=list(range(N_CORES)),
                               trace=_trace)
    outs = [r["out"].reshape(BL, T, 2) for r in res.results]
    full = np.concatenate(outs, axis=0).astype(np.float32)
    kernel.last_results = res
    return full


if __name__ == "__main__":
    pass
